# revision 10
# baseline (speedup 1.0000x reference)
"""Trainium2 Bass kernel: 3x3 conv (stride 1, pad 1) via shifted-matmul.

Full problem: x (32, 18, 256, 256) f32, weight (64, 18, 3, 3), bias (64,)
-> out (32, 64, 256, 256).  Data-parallel over batch: 8 cores x 4 images.

v2 design (paired partition halves, fp16):
  - All tensors fp16 on the wire: fp32 moving data streams through the PE
    at 2 cycles/col (measured 429 ns per N=512 matmul); fp16 streams at 1
    cycle/col (~213 ns) and halves every DMA byte count.  fp16 keeps
    ~2^-11 relative precision (rel err ~1e-4, same as the f32r baseline
    for this tolerance).
  - Strips of R=64 output rows alternate between SBUF partition halves:
    even strips (lo) hold G/weights on partitions 0..53, odd strips (hi)
    on 64..117.  Two wins:
      1. PE row-tiling: lo matmuls auto-derive tile_position (0,0), hi
         (64,64) -- disjoint row groups, so lo/hi matmuls execute
         CONCURRENTLY in the 128x128 array (each uses 54 rows x 64 cols).
      2. DMA port swizzle: partitions 0..63 map to the 8 even SBUF AXI
         ports, 64..127 to the 8 odd ports.  Alternating halves engages
         all 16 SDMA engines for loads and stores instead of 8.
  - Shared PSUM banks: one [128, 512] PSUM tile = lo row-pair (parts
    0..63) + hi row-pair (64..127).  has_written/pending-zero state is
    per-partition, so both halves run independent start/stop groups.
    Drains then run at full 128-lane width (ACT: identity+bias, DVE:
    tensor_scalar_add), writing fp16 into a [128, 4, 512] staging tile;
    two 256 KB stores per staging tile (one per half / port parity).
  - Matmuls issue in blocks of 2 banks per stationary (lo,lo,hi,hi) so
    walrus ldw-opt elides every second LDWEIGHTS and the other half's
    matmuls hide the remaining reloads.
"""

import re
import numpy as np

import bass_rust
import concourse.bass as bass
import concourse.mybir as mybir
from concourse.tile import TileContext


# ---------------------------------------------------------------------------
# TileContext drain patch: this walrus build rejects an InstDrain carrying
# more than ~2 sync waits ("Too many sync wait commands").  Re-emit the
# end-of-kernel global-clock waits as one nop per semaphore, then drain.
# ---------------------------------------------------------------------------
def _patched_drain_and_barrier(self, tick_clock, wait_clock):
    gc = tick_clock.global_clock
    vals = [int(s) for s in re.findall(r"\d+", repr(gc))]
    for i, v in enumerate(vals):
        if v > 0:
            c = bass_rust.VectorClock()
            c.require_at_least(i, v)
            nop = self.nc.sync.nop(nofuse=True, hint=f"drain_wait_{i}")
            wait_clock.add_sem_waits(nop.ins, bass_rust.ScopedClock({None: c}))
    self.nc.sync.drain()

    self.nc.all_engine_barrier()
    assert self.sems is not None
    popped = self.nc._tile_sem_poison_stack.pop()
    assert popped is self._sem_poison
    self.nc.clear_and_free_semaphores(list(self.sems.allocated().values()))
    self.nc.all_engine_barrier()


TileContext._drain_and_barrier = _patched_drain_and_barrier


def _patch_ldw_opt():
    """Re-enable walrus's load-weights elision (the pipeline passes
    --enable-ldw-opt=false) so weight-major matmul runs skip redundant
    stationary reloads."""
    import concourse.bass_utils as _bu

    if getattr(_bu, "_ldw_opt_patched", False):
        return
    _orig = _bu.run_command

    def _patched(cmd, *a, **kw):
        cmd = [
            "--enable-ldw-opt=true" if c == "--enable-ldw-opt=false" else c
            for c in cmd
        ]
        return _orig(cmd, *a, **kw)

    _bu.run_command = _patched
    _bu._ldw_opt_patched = True


def _split_excess_waits(nc, max_waits=1):
    """This walrus build allows very few sync waits per instruction.
    Hoist excess waits onto same-engine nops placed just before."""
    for f in nc.m.functions:
        for bb in f.blocks:
            out = []
            changed = False
            for inst in bb.instructions:
                si = inst.sync_info
                waits = list(si.on_wait) if si and si.on_wait else []
                if len(waits) > max_waits:
                    changed = True
                    extras, keep = waits[:-max_waits], waits[-max_waits:]
                    for j, w in enumerate(extras):
                        nop = mybir.InstNoOp(
                            name=f"{inst.name}_xw{j}", ins=[], outs=[]
                        )
                        nop.engine = inst.engine
                        nop.sync_info = mybir.SyncInfo(on_wait=[w], on_update=[])
                        out.append(nop)
                    inst.sync_info = mybir.SyncInfo(
                        on_wait=keep,
                        on_update=list(si.on_update) if si.on_update else [],
                    )
                out.append(inst)
            if changed:
                bb.instructions = out


# ---------------------------------------------------------------------------
# Kernel builder
# ---------------------------------------------------------------------------
F32 = mybir.dt.float32
F16 = mybir.dt.float16


def build_conv_nc(
    n_img=4,
    H=256,
    W=256,
    R=64,  # rows per strip; lo strip + hi strip = one super-strip
    C_IN=18,
    C_OUT=64,
    act_frac=4,  # of 8 drains, how many go to ACT (rest DVE)
    mm_block=2,  # banks per stationary before switching halves
    ob_banks=8,  # PSUM banks batched per output staging tile / store
):
    """Build the per-core Bass program. Returns nc."""
    assert H % (2 * R) == 0 and R % 16 == 0
    Wp = W + 2
    G_P = 3 * C_IN  # 54 partitions per half

    nc = bass.Bass()
    # x is host-pre-expanded into the exact per-strip SBUF layout:
    # x[s, 3c+g, r*Wp + w] = xpad[img(s), c, h0(s)+g+r, w].  Each strip
    # load is then a single fully-contiguous [54, R*Wp] transfer whose
    # outer dim (54) splits into 14 SDMA-engine chunks (vs 9 when the
    # source was [18 ch, 3 grp, run] -- the splitter chunks the outermost
    # dim only, ceil(18/16)=2 -> 9 chunks).
    n_strips = n_img * (H // R)
    x = nc.dram_tensor("x", [n_strips, G_P, R * Wp], F16, kind="ExternalInput")
    wT = nc.dram_tensor("wT", [G_P, 3, C_OUT], F16, kind="ExternalInput")
    bias2 = nc.dram_tensor("bias2", [2 * C_OUT, 1], F32, kind="ExternalInput")
    y = nc.dram_tensor("y", [n_img, C_OUT, H, W], F16, kind="ExternalOutput")

    n_super = H // (2 * R)
    pairs_per_strip = R // 2  # [*, 512] PSUM rows-pairs per strip
    n_rounds = pairs_per_strip // 8  # 8 shared banks per round
    assert (8 * 2) % ob_banks == 0
    x_ap = x[:]

    with TileContext(nc) as tc:
        with (
            tc.tile_pool(name="wpool", bufs=1) as wpool,
            tc.tile_pool(name="glo", bufs=2) as glo_pool,
            tc.tile_pool(name="ghi", bufs=2) as ghi_pool,
            tc.tile_pool(name="opool", bufs=4) as opool,
            tc.tile_pool(name="psum", bufs=8, space="PSUM") as pspool,
        ):
            wlo = wpool.tile([G_P, 3, C_OUT], F16, tag="wlo")
            whi_t = wpool.tile([64 + G_P, 3, C_OUT], F16, tag="whi")
            whi = whi_t[64 : 64 + G_P]
            bsb = wpool.tile([2 * C_OUT, 1], F32, tag="bsb")
            nc.sync.dma_start(out=wlo[:, :, :], in_=wT[:])
            nc.sync.dma_start(out=whi[:, :, :], in_=wT[:])
            nc.sync.dma_start(out=bsb[:], in_=bias2[:])

            tile_idx = 0
            for n in range(n_img):
                for ss in range(n_super):
                    hs = ss * 2 * R
                    Glo = glo_pool.tile([G_P, R, Wp], F16, tag="Glo")
                    Ghi_t = ghi_pool.tile([64 + G_P, R, Wp], F16, tag="Ghi")
                    Ghi = Ghi_t[64 : 64 + G_P]
                    # Partition p = 3c + g (channel-major); group g's window
                    # = padded-X rows [h0+g, h0+g+R), pre-gathered on host.
                    for half, dst in ((0, Glo), (1, Ghi)):
                        sid = (n * n_super + ss) * 2 + half
                        src = bass.AP(
                            tensor=x_ap.tensor,
                            offset=sid * G_P * R * Wp,
                            ap=[[R * Wp, G_P], [1, R * Wp]],
                        )
                        # Scalar (ACT) HWDGE queue: its transfers spread
                        # over all 16 SDMA engines; the sync queue's only
                        # ever reach 9.
                        nc.scalar.dma_start(out=dst[:, :, :], in_=src)

                    for rd in range(n_rounds):
                        PTs = [
                            pspool.tile(
                                [2 * C_OUT, 512], F32, tag="PT", name=f"PT{k}"
                            )
                            for k in range(8)
                        ]
                        # Matmuls: per bank-block, per tap: mm_block lo MMs
                        # (one stationary), then mm_block hi MMs.  lo and
                        # hi occupy disjoint PE row groups -> concurrent.
                        for pg in range(0, 8, mm_block):
                            ks = range(pg, pg + mm_block)
                            for t in range(3):
                                for k in ks:
                                    i = rd * 8 + k
                                    nc.tensor.matmul(
                                        PTs[k][0:C_OUT],
                                        wlo[:, t, :],
                                        Glo[:, 2 * i : 2 * i + 2, t : t + W],
                                        start=(t == 0),
                                        stop=(t == 2),
                                        skip_group_check=True,
                                    )
                                for k in ks:
                                    i = rd * 8 + k
                                    nc.tensor.matmul(
                                        PTs[k][C_OUT : 2 * C_OUT],
                                        whi[:, t, :],
                                        Ghi[:, 2 * i : 2 * i + 2, t : t + W],
                                        start=(t == 0),
                                        stop=(t == 2),
                                        skip_group_check=True,
                                    )
                        # Drain ob_banks banks into one [128, ob_banks, 512]
                        # fp16 staging tile; store each half (2*ob_banks rows
                        # x 64 oc) as one contiguous-per-partition transfer.
                        for ob_i in range(8 // ob_banks):
                            OB = opool.tile(
                                [2 * C_OUT, ob_banks, 512], F16, tag="OB"
                            )
                            for u in range(ob_banks):
                                PT = PTs[ob_i * ob_banks + u]
                                if tile_idx % 8 < act_frac:
                                    nc.scalar.activation(
                                        OB[:, u, :],
                                        PT[:],
                                        mybir.ActivationFunctionType.Identity,
                                        bias=bsb[0 : 2 * C_OUT],
                                    )
                                else:
                                    nc.vector.tensor_scalar_add(
                                        OB[:, u, :], PT[:], bsb[0 : 2 * C_OUT]
                                    )
                                tile_idx += 1
                            nr = 2 * ob_banks
                            h_lo = hs + rd * 16 + ob_i * nr
                            h_hi = h_lo + R
                            nc.scalar.dma_start(
                                out=y[n, :, h_lo : h_lo + nr, :],
                                in_=OB[0:C_OUT],
                            )
                            nc.scalar.dma_start(
                                out=y[n, :, h_hi : h_hi + nr, :],
                                in_=OB[C_OUT : 2 * C_OUT],
                            )
    return nc


# ---------------------------------------------------------------------------
# Host-side entry point
# ---------------------------------------------------------------------------
N_CORES = 8


def prep_inputs(x_shard, weight, bias, R=64):
    # lhsT row 3c+g = weight[:, c, g, b]; lhsT col = oc
    wT = np.ascontiguousarray(
        np.transpose(weight, (1, 2, 3, 0)).reshape(54, 3, 64)
    ).astype(np.float16)
    bias2 = np.concatenate([bias, bias]).reshape(128, 1).astype(np.float32)
    n, c, H, W = x_shard.shape
    Wp = W + 2
    x_pad = np.zeros((n, c, H + 2, Wp), np.float16)
    x_pad[:, :, 1 : H + 1, 1 : W + 1] = x_shard
    # Strip-expand into the exact SBUF layout (see build_conv_nc): strip
    # sid covers output rows [h0, h0+R); partition 3c+g holds padded rows
    # [h0+g, h0+g+R).  Strips are ordered (img, super, half).
    spi = H // R  # strips per image
    xs = np.empty((n * spi, 54, R * Wp), np.float16)
    for img in range(n):
        for s in range(spi):
            h0 = s * R
            # [c, g, r, w] -> [54, R*Wp]
            win = np.stack(
                [x_pad[img, :, h0 + g : h0 + g + R, :] for g in range(3)],
                axis=1,
            )
            xs[img * spi + s] = win.reshape(54, R * Wp)
    return {"x": xs, "wT": wT, "bias2": bias2}


def run(x, weight, bias, trace=False, **build_kwargs):
    from concourse.bass_utils import run_bass_kernel_spmd

    x = np.asarray(x, dtype=np.float32)
    weight = np.asarray(weight, dtype=np.float32)
    bias = np.asarray(bias, dtype=np.float32)

    B = x.shape[0]
    per = B // N_CORES
    nc = build_conv_nc(n_img=per, **build_kwargs)
    _split_excess_waits(nc)
    # NOTE: walrus ldw-opt (weight-reload elision) rejects LDWEIGHTS with
    # col-tiling (tile_position=(64,64)), so it stays at the pipeline
    # default (off).  The per-matmul 53 ns weight load overlaps the other
    # partition half's matmuls.
    in_maps = [
        prep_inputs(x[i * per : (i + 1) * per], weight, bias)
        for i in range(N_CORES)
    ]
    res = run_bass_kernel_spmd(nc, in_maps, list(range(N_CORES)), trace=trace)
    y = np.concatenate(
        [res.results[i]["y"] for i in range(N_CORES)], axis=0
    ).astype(np.float32)
    return y, res


def kernel(x, weight, bias):
    return run(x, weight, bias)[0]


# revision 11
# speedup vs baseline: 1.3917x; 1.3917x over previous
"""Trainium2 Bass kernel: 3x3 conv (stride 1, pad 1) via shifted-matmul.

Full problem: x (32, 18, 256, 256) f32, weight (64, 18, 3, 3), bias (64,)
-> out (32, 64, 256, 256).  Data-parallel over batch: 8 cores x 4 images.

v2 design (paired partition halves, fp16):
  - All tensors fp16 on the wire: fp32 moving data streams through the PE
    at 2 cycles/col (measured 429 ns per N=512 matmul); fp16 streams at 1
    cycle/col (~213 ns) and halves every DMA byte count.  fp16 keeps
    ~2^-11 relative precision (rel err ~1e-4, same as the f32r baseline
    for this tolerance).
  - Strips of R=64 output rows alternate between SBUF partition halves:
    even strips (lo) hold G/weights on partitions 0..53, odd strips (hi)
    on 64..117.  Two wins:
      1. PE row-tiling: lo matmuls auto-derive tile_position (0,0), hi
         (64,64) -- disjoint row groups, so lo/hi matmuls execute
         CONCURRENTLY in the 128x128 array (each uses 54 rows x 64 cols).
      2. DMA port swizzle: partitions 0..63 map to the 8 even SBUF AXI
         ports, 64..127 to the 8 odd ports.  Alternating halves engages
         all 16 SDMA engines for loads and stores instead of 8.
  - Shared PSUM banks: one [128, 512] PSUM tile = lo row-pair (parts
    0..63) + hi row-pair (64..127).  has_written/pending-zero state is
    per-partition, so both halves run independent start/stop groups.
    Drains then run at full 128-lane width (ACT: identity+bias, DVE:
    tensor_scalar_add), writing fp16 into a [128, 4, 512] staging tile;
    two 256 KB stores per staging tile (one per half / port parity).
  - Matmuls issue in blocks of 2 banks per stationary (lo,lo,hi,hi) so
    walrus ldw-opt elides every second LDWEIGHTS and the other half's
    matmuls hide the remaining reloads.
"""

import re
import numpy as np

import bass_rust
import concourse.bass as bass
import concourse.mybir as mybir
from concourse.tile import TileContext


# ---------------------------------------------------------------------------
# TileContext drain patch: this walrus build rejects an InstDrain carrying
# more than ~2 sync waits ("Too many sync wait commands").  Re-emit the
# end-of-kernel global-clock waits as one nop per semaphore, then drain.
# ---------------------------------------------------------------------------
def _patched_drain_and_barrier(self, tick_clock, wait_clock):
    gc = tick_clock.global_clock
    vals = [int(s) for s in re.findall(r"\d+", repr(gc))]
    for i, v in enumerate(vals):
        if v > 0:
            c = bass_rust.VectorClock()
            c.require_at_least(i, v)
            nop = self.nc.sync.nop(nofuse=True, hint=f"drain_wait_{i}")
            wait_clock.add_sem_waits(nop.ins, bass_rust.ScopedClock({None: c}))
    self.nc.sync.drain()

    self.nc.all_engine_barrier()
    assert self.sems is not None
    popped = self.nc._tile_sem_poison_stack.pop()
    assert popped is self._sem_poison
    self.nc.clear_and_free_semaphores(list(self.sems.allocated().values()))
    self.nc.all_engine_barrier()


TileContext._drain_and_barrier = _patched_drain_and_barrier


def _patch_ldw_opt():
    """Re-enable walrus's load-weights elision (the pipeline passes
    --enable-ldw-opt=false) so weight-major matmul runs skip redundant
    stationary reloads."""
    import concourse.bass_utils as _bu

    if getattr(_bu, "_ldw_opt_patched", False):
        return
    _orig = _bu.run_command

    def _patched(cmd, *a, **kw):
        cmd = [
            "--enable-ldw-opt=true" if c == "--enable-ldw-opt=false" else c
            for c in cmd
        ]
        return _orig(cmd, *a, **kw)

    _bu.run_command = _patched
    _bu._ldw_opt_patched = True


def _split_excess_waits(nc, max_waits=1):
    """This walrus build allows very few sync waits per instruction.
    Hoist excess waits onto same-engine nops placed just before."""
    for f in nc.m.functions:
        for bb in f.blocks:
            out = []
            changed = False
            for inst in bb.instructions:
                si = inst.sync_info
                waits = list(si.on_wait) if si and si.on_wait else []
                if len(waits) > max_waits:
                    changed = True
                    extras, keep = waits[:-max_waits], waits[-max_waits:]
                    for j, w in enumerate(extras):
                        nop = mybir.InstNoOp(
                            name=f"{inst.name}_xw{j}", ins=[], outs=[]
                        )
                        nop.engine = inst.engine
                        nop.sync_info = mybir.SyncInfo(on_wait=[w], on_update=[])
                        out.append(nop)
                    inst.sync_info = mybir.SyncInfo(
                        on_wait=keep,
                        on_update=list(si.on_update) if si.on_update else [],
                    )
                out.append(inst)
            if changed:
                bb.instructions = out


# ---------------------------------------------------------------------------
# Kernel builder
# ---------------------------------------------------------------------------
F32 = mybir.dt.float32
F16 = mybir.dt.float16


def build_conv_nc(
    n_img=4,
    H=256,
    W=256,
    R=64,  # rows per strip; lo strip + hi strip = one super-strip
    C_IN=18,
    C_OUT=64,
    act_frac=4,  # of 8 drains, how many go to ACT (rest DVE)
    mm_block=2,  # banks per stationary before switching halves
    ob_banks=8,  # PSUM banks batched per output staging tile / store
):
    """Build the per-core Bass program. Returns nc."""
    assert H % (2 * R) == 0 and R % 16 == 0
    Wp = W + 2
    G_P = 3 * C_IN  # 54 partitions per half

    nc = bass.Bass()
    # x is host-pre-expanded into the exact per-strip SBUF layout:
    # x[s, 3c+g, r*Wp + w] = xpad[img(s), c, h0(s)+g+r, w].  Each strip
    # load is then a single fully-contiguous [54, R*Wp] transfer whose
    # outer dim (54) splits into 14 SDMA-engine chunks (vs 9 when the
    # source was [18 ch, 3 grp, run] -- the splitter chunks the outermost
    # dim only, ceil(18/16)=2 -> 9 chunks).
    n_strips = n_img * (H // R)
    x = nc.dram_tensor("x", [n_strips, G_P, R * Wp], F16, kind="ExternalInput")
    wT = nc.dram_tensor("wT", [G_P, 3, C_OUT], F16, kind="ExternalInput")
    bias2 = nc.dram_tensor("bias2", [2 * C_OUT, 1], F32, kind="ExternalInput")
    y = nc.dram_tensor("y", [n_img, C_OUT, H, W], F16, kind="ExternalOutput")

    n_super = H // (2 * R)
    pairs_per_strip = R // 2  # [*, 512] PSUM rows-pairs per strip
    n_rounds = pairs_per_strip // 8  # 8 shared banks per round
    assert (8 * 2) % ob_banks == 0
    x_ap = x[:]

    with TileContext(nc) as tc:
        with (
            tc.tile_pool(name="wpool", bufs=1) as wpool,
            tc.tile_pool(name="glo", bufs=2) as glo_pool,
            tc.tile_pool(name="ghi", bufs=2) as ghi_pool,
            tc.tile_pool(name="opool", bufs=4) as opool,
            tc.tile_pool(name="psum", bufs=8, space="PSUM") as pspool,
        ):
            wlo = wpool.tile([G_P, 3, C_OUT], F16, tag="wlo")
            whi_t = wpool.tile([64 + G_P, 3, C_OUT], F16, tag="whi")
            whi = whi_t[64 : 64 + G_P]
            bsb = wpool.tile([2 * C_OUT, 1], F32, tag="bsb")
            nc.sync.dma_start(out=wlo[:, :, :], in_=wT[:])
            nc.sync.dma_start(out=whi[:, :, :], in_=wT[:])
            nc.sync.dma_start(out=bsb[:], in_=bias2[:])

            tile_idx = 0
            for n in range(n_img):
                for ss in range(n_super):
                    hs = ss * 2 * R
                    Glo = glo_pool.tile([G_P, R, Wp], F16, tag="Glo")
                    Ghi_t = ghi_pool.tile([64 + G_P, R, Wp], F16, tag="Ghi")
                    Ghi = Ghi_t[64 : 64 + G_P]
                    # Partition p = 3c + g (channel-major); group g's window
                    # = padded-X rows [h0+g, h0+g+R), pre-gathered on host.
                    for half, dst in ((0, Glo), (1, Ghi)):
                        sid = (n * n_super + ss) * 2 + half
                        src = bass.AP(
                            tensor=x_ap.tensor,
                            offset=sid * G_P * R * Wp,
                            ap=[[R * Wp, G_P], [1, R * Wp]],
                        )
                        # HWDGE loads (sync or scalar queue) only ever
                        # split across 9 SDMA engines; SWDGE (gpsimd)
                        # assigns descriptors by destination-partition
                        # port, so lo (even ports) + hi (odd ports)
                        # engage all 16 engines.
                        nc.gpsimd.dma_start(out=dst[:, :, :], in_=src)

                    for rd in range(n_rounds):
                        PTs = [
                            pspool.tile(
                                [2 * C_OUT, 512], F32, tag="PT", name=f"PT{k}"
                            )
                            for k in range(8)
                        ]
                        # Matmuls: per bank-block, per tap: mm_block lo MMs
                        # (one stationary), then mm_block hi MMs.  lo and
                        # hi occupy disjoint PE row groups -> concurrent.
                        for pg in range(0, 8, mm_block):
                            ks = range(pg, pg + mm_block)
                            for t in range(3):
                                for k in ks:
                                    i = rd * 8 + k
                                    nc.tensor.matmul(
                                        PTs[k][0:C_OUT],
                                        wlo[:, t, :],
                                        Glo[:, 2 * i : 2 * i + 2, t : t + W],
                                        start=(t == 0),
                                        stop=(t == 2),
                                        skip_group_check=True,
                                    )
                                for k in ks:
                                    i = rd * 8 + k
                                    nc.tensor.matmul(
                                        PTs[k][C_OUT : 2 * C_OUT],
                                        whi[:, t, :],
                                        Ghi[:, 2 * i : 2 * i + 2, t : t + W],
                                        start=(t == 0),
                                        stop=(t == 2),
                                        skip_group_check=True,
                                    )
                        # Drain ob_banks banks into one [128, ob_banks, 512]
                        # fp16 staging tile; store each half (2*ob_banks rows
                        # x 64 oc) as one contiguous-per-partition transfer.
                        for ob_i in range(8 // ob_banks):
                            OB = opool.tile(
                                [2 * C_OUT, ob_banks, 512], F16, tag="OB"
                            )
                            for u in range(ob_banks):
                                PT = PTs[ob_i * ob_banks + u]
                                if tile_idx % 8 < act_frac:
                                    nc.scalar.activation(
                                        OB[:, u, :],
                                        PT[:],
                                        mybir.ActivationFunctionType.Identity,
                                        bias=bsb[0 : 2 * C_OUT],
                                    )
                                else:
                                    nc.vector.tensor_scalar_add(
                                        OB[:, u, :], PT[:], bsb[0 : 2 * C_OUT]
                                    )
                                tile_idx += 1
                            nr = 2 * ob_banks
                            h_lo = hs + rd * 16 + ob_i * nr
                            h_hi = h_lo + R
                            nc.scalar.dma_start(
                                out=y[n, :, h_lo : h_lo + nr, :],
                                in_=OB[0:C_OUT],
                            )
                            nc.scalar.dma_start(
                                out=y[n, :, h_hi : h_hi + nr, :],
                                in_=OB[C_OUT : 2 * C_OUT],
                            )
    return nc


# ---------------------------------------------------------------------------
# Host-side entry point
# ---------------------------------------------------------------------------
N_CORES = 8


def prep_inputs(x_shard, weight, bias, R=64):
    # lhsT row 3c+g = weight[:, c, g, b]; lhsT col = oc
    wT = np.ascontiguousarray(
        np.transpose(weight, (1, 2, 3, 0)).reshape(54, 3, 64)
    ).astype(np.float16)
    bias2 = np.concatenate([bias, bias]).reshape(128, 1).astype(np.float32)
    n, c, H, W = x_shard.shape
    Wp = W + 2
    x_pad = np.zeros((n, c, H + 2, Wp), np.float16)
    x_pad[:, :, 1 : H + 1, 1 : W + 1] = x_shard
    # Strip-expand into the exact SBUF layout (see build_conv_nc): strip
    # sid covers output rows [h0, h0+R); partition 3c+g holds padded rows
    # [h0+g, h0+g+R).  Strips are ordered (img, super, half).
    spi = H // R  # strips per image
    xs = np.empty((n * spi, 54, R * Wp), np.float16)
    for img in range(n):
        for s in range(spi):
            h0 = s * R
            # [c, g, r, w] -> [54, R*Wp]
            win = np.stack(
                [x_pad[img, :, h0 + g : h0 + g + R, :] for g in range(3)],
                axis=1,
            )
            xs[img * spi + s] = win.reshape(54, R * Wp)
    return {"x": xs, "wT": wT, "bias2": bias2}


def run(x, weight, bias, trace=False, **build_kwargs):
    from concourse.bass_utils import run_bass_kernel_spmd

    x = np.asarray(x, dtype=np.float32)
    weight = np.asarray(weight, dtype=np.float32)
    bias = np.asarray(bias, dtype=np.float32)

    B = x.shape[0]
    per = B // N_CORES
    nc = build_conv_nc(n_img=per, **build_kwargs)
    _split_excess_waits(nc)
    # NOTE: walrus ldw-opt (weight-reload elision) rejects LDWEIGHTS with
    # col-tiling (tile_position=(64,64)), so it stays at the pipeline
    # default (off).  The per-matmul 53 ns weight load overlaps the other
    # partition half's matmuls.
    in_maps = [
        prep_inputs(x[i * per : (i + 1) * per], weight, bias)
        for i in range(N_CORES)
    ]
    res = run_bass_kernel_spmd(nc, in_maps, list(range(N_CORES)), trace=trace)
    y = np.concatenate(
        [res.results[i]["y"] for i in range(N_CORES)], axis=0
    ).astype(np.float32)
    return y, res


def kernel(x, weight, bias):
    return run(x, weight, bias)[0]


# revision 13
# speedup vs baseline: 1.4772x; 1.0614x over previous
"""Trainium2 Bass kernel: 3x3 conv (stride 1, pad 1) via shifted-matmul.

Full problem: x (32, 18, 256, 256) f32, weight (64, 18, 3, 3), bias (64,)
-> out (32, 64, 256, 256).  Data-parallel over batch: 8 cores x 4 images.

v2 design (paired partition halves, fp16):
  - All tensors fp16 on the wire: fp32 moving data streams through the PE
    at 2 cycles/col (measured 429 ns per N=512 matmul); fp16 streams at 1
    cycle/col (~213 ns) and halves every DMA byte count.  fp16 keeps
    ~2^-11 relative precision (rel err ~1e-4, same as the f32r baseline
    for this tolerance).
  - Strips of R=64 output rows alternate between SBUF partition halves:
    even strips (lo) hold G/weights on partitions 0..53, odd strips (hi)
    on 64..117.  Two wins:
      1. PE row-tiling: lo matmuls auto-derive tile_position (0,0), hi
         (64,64) -- disjoint row groups, so lo/hi matmuls execute
         CONCURRENTLY in the 128x128 array (each uses 54 rows x 64 cols).
      2. DMA port swizzle: partitions 0..63 map to the 8 even SBUF AXI
         ports, 64..127 to the 8 odd ports.  Alternating halves engages
         all 16 SDMA engines for loads and stores instead of 8.
  - Shared PSUM banks: one [128, 512] PSUM tile = lo row-pair (parts
    0..63) + hi row-pair (64..127).  has_written/pending-zero state is
    per-partition, so both halves run independent start/stop groups.
    Drains then run at full 128-lane width (ACT: identity+bias, DVE:
    tensor_scalar_add), writing fp16 into a [128, 4, 512] staging tile;
    two 256 KB stores per staging tile (one per half / port parity).
  - Matmuls issue in blocks of 2 banks per stationary (lo,lo,hi,hi) so
    walrus ldw-opt elides every second LDWEIGHTS and the other half's
    matmuls hide the remaining reloads.
"""

import re
import numpy as np

import bass_rust
import concourse.bass as bass
import concourse.mybir as mybir
from concourse.tile import TileContext


# ---------------------------------------------------------------------------
# TileContext drain patch: this walrus build rejects an InstDrain carrying
# more than ~2 sync waits ("Too many sync wait commands").  Re-emit the
# end-of-kernel global-clock waits as one nop per semaphore, then drain.
# ---------------------------------------------------------------------------
def _patched_drain_and_barrier(self, tick_clock, wait_clock):
    gc = tick_clock.global_clock
    vals = [int(s) for s in re.findall(r"\d+", repr(gc))]
    for i, v in enumerate(vals):
        if v > 0:
            c = bass_rust.VectorClock()
            c.require_at_least(i, v)
            nop = self.nc.sync.nop(nofuse=True, hint=f"drain_wait_{i}")
            wait_clock.add_sem_waits(nop.ins, bass_rust.ScopedClock({None: c}))
    self.nc.sync.drain()

    self.nc.all_engine_barrier()
    assert self.sems is not None
    popped = self.nc._tile_sem_poison_stack.pop()
    assert popped is self._sem_poison
    self.nc.clear_and_free_semaphores(list(self.sems.allocated().values()))
    self.nc.all_engine_barrier()


TileContext._drain_and_barrier = _patched_drain_and_barrier


def _patch_ldw_opt():
    """Re-enable walrus's load-weights elision (the pipeline passes
    --enable-ldw-opt=false) so weight-major matmul runs skip redundant
    stationary reloads."""
    import concourse.bass_utils as _bu

    if getattr(_bu, "_ldw_opt_patched", False):
        return
    _orig = _bu.run_command

    def _patched(cmd, *a, **kw):
        cmd = [
            "--enable-ldw-opt=true" if c == "--enable-ldw-opt=false" else c
            for c in cmd
        ]
        return _orig(cmd, *a, **kw)

    _bu.run_command = _patched
    _bu._ldw_opt_patched = True


def _split_excess_waits(nc, max_waits=1):
    """This walrus build allows very few sync waits per instruction.
    Hoist excess waits onto same-engine nops placed just before."""
    for f in nc.m.functions:
        for bb in f.blocks:
            out = []
            changed = False
            for inst in bb.instructions:
                si = inst.sync_info
                waits = list(si.on_wait) if si and si.on_wait else []
                if len(waits) > max_waits:
                    changed = True
                    extras, keep = waits[:-max_waits], waits[-max_waits:]
                    for j, w in enumerate(extras):
                        nop = mybir.InstNoOp(
                            name=f"{inst.name}_xw{j}", ins=[], outs=[]
                        )
                        nop.engine = inst.engine
                        nop.sync_info = mybir.SyncInfo(on_wait=[w], on_update=[])
                        out.append(nop)
                    inst.sync_info = mybir.SyncInfo(
                        on_wait=keep,
                        on_update=list(si.on_update) if si.on_update else [],
                    )
                out.append(inst)
            if changed:
                bb.instructions = out


# ---------------------------------------------------------------------------
# Kernel builder
# ---------------------------------------------------------------------------
F32 = mybir.dt.float32
F16 = mybir.dt.float16


def build_conv_nc(
    n_img=4,
    H=256,
    W=256,
    R=64,  # rows per strip; lo strip + hi strip = one super-strip
    C_IN=18,
    C_OUT=64,
    act_frac=4,  # of 8 drains, how many go to ACT (rest DVE)
    mm_block=2,  # banks per stationary before switching halves
    ob_banks=8,  # PSUM banks batched per output staging tile / store
):
    """Build the per-core Bass program. Returns nc."""
    assert H % (2 * R) == 0 and R % 16 == 0
    Wp = W + 2
    G_P = 3 * C_IN  # 54 partitions per half

    nc = bass.Bass()
    # x is host-pre-expanded into the exact per-strip SBUF layout:
    # x[s, 3c+g, r*Wp + w] = xpad[img(s), c, h0(s)+g+r, w].  Each strip
    # load is then a single fully-contiguous [54, R*Wp] transfer whose
    # outer dim (54) splits into 14 SDMA-engine chunks (vs 9 when the
    # source was [18 ch, 3 grp, run] -- the splitter chunks the outermost
    # dim only, ceil(18/16)=2 -> 9 chunks).
    n_strips = n_img * (H // R)
    x = nc.dram_tensor("x", [n_strips, G_P, R * Wp], F16, kind="ExternalInput")
    wT = nc.dram_tensor("wT", [G_P, 3, C_OUT], F16, kind="ExternalInput")
    bias2 = nc.dram_tensor("bias2", [2 * C_OUT, 1], F32, kind="ExternalInput")
    y = nc.dram_tensor("y", [n_img, C_OUT, H, W], F16, kind="ExternalOutput")

    n_super = H // (2 * R)
    pairs_per_strip = R // 2  # [*, 512] PSUM rows-pairs per strip
    n_rounds = pairs_per_strip // 8  # 8 shared banks per round
    assert (8 * 2) % ob_banks == 0
    x_ap = x[:]

    with TileContext(nc) as tc:
        with (
            tc.tile_pool(name="wpool", bufs=1) as wpool,
            tc.tile_pool(name="glo", bufs=2) as glo_pool,
            tc.tile_pool(name="ghi", bufs=2) as ghi_pool,
            tc.tile_pool(name="opool", bufs=4) as opool,
            tc.tile_pool(name="psum", bufs=8, space="PSUM") as pspool,
        ):
            wlo = wpool.tile([G_P, 3, C_OUT], F16, tag="wlo")
            whi_t = wpool.tile([64 + G_P, 3, C_OUT], F16, tag="whi")
            whi = whi_t[64 : 64 + G_P]
            bsb = wpool.tile([2 * C_OUT, 1], F32, tag="bsb")
            nc.sync.dma_start(out=wlo[:, :, :], in_=wT[:])
            nc.sync.dma_start(out=whi[:, :, :], in_=wT[:])
            nc.sync.dma_start(out=bsb[:], in_=bias2[:])

            tile_idx = 0
            for n in range(n_img):
                for ss in range(n_super):
                    hs = ss * 2 * R
                    Glo = glo_pool.tile([G_P, R, Wp], F16, tag="Glo")
                    Ghi_t = ghi_pool.tile([64 + G_P, R, Wp], F16, tag="Ghi")
                    Ghi = Ghi_t[64 : 64 + G_P]
                    # Partition p = 3c + g (channel-major); group g's window
                    # = padded-X rows [h0+g, h0+g+R), pre-gathered on host.
                    # HWDGE loads (sync or scalar queue) only ever split
                    # across 9 SDMA engines; SWDGE (gpsimd) assigns
                    # descriptors by destination-partition port, so lo
                    # (even ports) + hi (odd ports) engage all 16 engines.
                    # Each strip loads as two half-strip chunks so the
                    # first rounds' matmuls start after ~0.9 MB, not 1.8.
                    Rh = R // 2
                    for half, dst in ((0, Glo), (1, Ghi)):
                        sid = (n * n_super + ss) * 2 + half
                        for ch in range(2):
                            src = bass.AP(
                                tensor=x_ap.tensor,
                                offset=sid * G_P * R * Wp + ch * Rh * Wp,
                                ap=[[R * Wp, G_P], [1, Rh * Wp]],
                            )
                            nc.gpsimd.dma_start(
                                out=dst[:, ch * Rh : (ch + 1) * Rh, :],
                                in_=src,
                            )

                    for rd2 in range(n_rounds // 2):
                        # One [128, 16, 512] fp16 staging tile spans two
                        # 8-bank rounds -> 1 MB stores (16 KB contiguous
                        # per oc partition).
                        OB = opool.tile([2 * C_OUT, 16, 512], F16, tag="OB")
                        for sub in range(2):
                            rd = rd2 * 2 + sub
                            PTs = [
                                pspool.tile(
                                    [2 * C_OUT, 512],
                                    F32,
                                    tag="PT",
                                    name=f"PT{k}",
                                )
                                for k in range(8)
                            ]
                            # Matmuls: per bank-block, per tap: mm_block lo
                            # MMs (one stationary), then mm_block hi MMs.
                            # lo and hi occupy disjoint PE row groups ->
                            # concurrent execution in the 128x128 array.
                            for pg in range(0, 8, mm_block):
                                ks = range(pg, pg + mm_block)
                                for t in range(3):
                                    for k in ks:
                                        i = rd * 8 + k
                                        nc.tensor.matmul(
                                            PTs[k][0:C_OUT],
                                            wlo[:, t, :],
                                            Glo[
                                                :, 2 * i : 2 * i + 2, t : t + W
                                            ],
                                            start=(t == 0),
                                            stop=(t == 2),
                                            skip_group_check=True,
                                        )
                                    for k in ks:
                                        i = rd * 8 + k
                                        nc.tensor.matmul(
                                            PTs[k][C_OUT : 2 * C_OUT],
                                            whi[:, t, :],
                                            Ghi[
                                                :, 2 * i : 2 * i + 2, t : t + W
                                            ],
                                            start=(t == 0),
                                            stop=(t == 2),
                                            skip_group_check=True,
                                        )
                            for k in range(8):
                                PT = PTs[k]
                                u = sub * 8 + k
                                if tile_idx % 8 < act_frac:
                                    nc.scalar.activation(
                                        OB[:, u, :],
                                        PT[:],
                                        mybir.ActivationFunctionType.Identity,
                                        bias=bsb[0 : 2 * C_OUT],
                                    )
                                else:
                                    nc.vector.tensor_scalar_add(
                                        OB[:, u, :], PT[:], bsb[0 : 2 * C_OUT]
                                    )
                                tile_idx += 1
                        h_lo = hs + rd2 * 32
                        h_hi = h_lo + R
                        nc.scalar.dma_start(
                            out=y[n, :, h_lo : h_lo + 32, :],
                            in_=OB[0:C_OUT],
                        )
                        nc.scalar.dma_start(
                            out=y[n, :, h_hi : h_hi + 32, :],
                            in_=OB[C_OUT : 2 * C_OUT],
                        )
    return nc


# ---------------------------------------------------------------------------
# Host-side entry point
# ---------------------------------------------------------------------------
N_CORES = 8


def prep_inputs(x_shard, weight, bias, R=64):
    # lhsT row 3c+g = weight[:, c, g, b]; lhsT col = oc
    wT = np.ascontiguousarray(
        np.transpose(weight, (1, 2, 3, 0)).reshape(54, 3, 64)
    ).astype(np.float16)
    bias2 = np.concatenate([bias, bias]).reshape(128, 1).astype(np.float32)
    n, c, H, W = x_shard.shape
    Wp = W + 2
    x_pad = np.zeros((n, c, H + 2, Wp), np.float16)
    x_pad[:, :, 1 : H + 1, 1 : W + 1] = x_shard
    # Strip-expand into the exact SBUF layout (see build_conv_nc): strip
    # sid covers output rows [h0, h0+R); partition 3c+g holds padded rows
    # [h0+g, h0+g+R).  Strips are ordered (img, super, half).
    spi = H // R  # strips per image
    xs = np.empty((n * spi, 54, R * Wp), np.float16)
    for img in range(n):
        for s in range(spi):
            h0 = s * R
            # [c, g, r, w] -> [54, R*Wp]
            win = np.stack(
                [x_pad[img, :, h0 + g : h0 + g + R, :] for g in range(3)],
                axis=1,
            )
            xs[img * spi + s] = win.reshape(54, R * Wp)
    return {"x": xs, "wT": wT, "bias2": bias2}


def run(x, weight, bias, trace=False, **build_kwargs):
    from concourse.bass_utils import run_bass_kernel_spmd

    x = np.asarray(x, dtype=np.float32)
    weight = np.asarray(weight, dtype=np.float32)
    bias = np.asarray(bias, dtype=np.float32)

    B = x.shape[0]
    per = B // N_CORES
    nc = build_conv_nc(n_img=per, **build_kwargs)
    _split_excess_waits(nc)
    # NOTE: walrus ldw-opt (weight-reload elision) rejects LDWEIGHTS with
    # col-tiling (tile_position=(64,64)), so it stays at the pipeline
    # default (off).  The per-matmul 53 ns weight load overlaps the other
    # partition half's matmuls.
    in_maps = [
        prep_inputs(x[i * per : (i + 1) * per], weight, bias)
        for i in range(N_CORES)
    ]
    res = run_bass_kernel_spmd(nc, in_maps, list(range(N_CORES)), trace=trace)
    y = np.concatenate(
        [res.results[i]["y"] for i in range(N_CORES)], axis=0
    ).astype(np.float32)
    return y, res


def kernel(x, weight, bias):
    return run(x, weight, bias)[0]
